# revision 29
# baseline (speedup 1.0000x reference)
"""Trainium2 Bass kernel for biased multi-head attention with sigmoid gating.

Problem (B=2, N=2048, C_IN=256, H=8, C_H=32):
    q = (q_x @ Wq) / sqrt(C_H);  k = kv_x @ Wk;  v = kv_x @ Wv
    a = softmax(q k^T + bias);   o = (a v) * sigmoid(q_x @ Wg + bg)
    out = o @ Wo + bo

Sharding: 8 cores, each takes (batch b = core//4, head pair hp = core%4).
Per core the kernel computes, for its 2 heads, the *unnormalized* gated
attention output projected through Wo, plus the softmax denominators; the
host divides by the denominators, sums partials over head-pairs, and adds bo.

Device-side layout highlights:
  - everything enters the PE in float16 (1 cycle/row vs 4 for fp32)
  - x and bias arrive host-pre-transposed f16, so no on-chip transposes
  - scores are built transposed [k, q] in PSUM: a K=128 zero-padded QK^T
    matmul (full-density contraction keeps the PE HAM activity monitor
    happy -- K<128 matmuls don't count as "busy" and the PE clock drops
    from 2.4 to 1.2 GHz), then the bias tile is accumulated into the same
    PSUM bank by an identity-weight matmul
  - softmax denominator comes free from a ones-column appended to V
  - exp runs on ScalarE straight out of PSUM, writing f16 probs to SBUF
  - the two 1024-wide q-chunks' AV matmuls are column-tiled to PE
    partition bases 0 and 64 and run concurrently; Wo is duplicated at
    both row bands so one K=128 projection covers both lanes
"""

import math
import sys

import numpy as np

sys.path.insert(0, "/opt/trn_rl_repo")

import concourse.bass as bass  # noqa: E402
import concourse.mybir as mybir  # noqa: E402
import concourse.tile as tile  # noqa: E402
from concourse import bacc  # noqa: E402
from concourse.masks import make_identity  # noqa: E402

B, N, C_IN = 2, 2048, 256
H, C_H = 8, 32
P = 128
NH_LOC = 2  # heads per core
QW = 1024  # q-chunk width in the main loop
KC = N // P  # 16 k-chunks per head
V_SCALE = 1.0 / 64.0  # keeps unnormalized (exp @ V) in f16 range; cancels on host
F32 = mybir.dt.float32
F16 = mybir.dt.float16


def build_nc():
    nc = bacc.Bacc("TRN2", target_bir_lowering=False, debug=False)

    xqT_d = nc.dram_tensor("xqT", [C_IN, N], F16, kind="ExternalInput")
    xkvT_d = nc.dram_tensor("xkvT", [C_IN, N], F16, kind="ExternalInput")
    bias_d = nc.dram_tensor("biasf", [NH_LOC, N, N], F16, kind="ExternalInput")
    wq_d = nc.dram_tensor("wq", [C_IN, 2 * C_H], F16, kind="ExternalInput")
    wk_d = nc.dram_tensor("wk", [C_IN, 2 * C_H], F16, kind="ExternalInput")
    wv_d = nc.dram_tensor("wv", [C_IN, 2 * C_H], F16, kind="ExternalInput")
    wg_d = nc.dram_tensor("wg", [C_IN, 2 * C_H], F16, kind="ExternalInput")
    wo_d = nc.dram_tensor("wo", [2 * C_H, C_IN], F16, kind="ExternalInput")
    bg_d = nc.dram_tensor("bg", [2 * C_H], F32, kind="ExternalInput")
    outp_d = nc.dram_tensor("outp", [NH_LOC, 2, P, N], F16, kind="ExternalOutput")
    sums_d = nc.dram_tensor("sums", [1, NH_LOC, N], F32, kind="ExternalOutput")

    with tile.TileContext(nc) as tc:
        with (
            tc.tile_pool(name="const", bufs=1) as const,
            tc.tile_pool(name="work", bufs=3) as work,
            tc.tile_pool(name="pbig", bufs=2, space="PSUM") as pbig,
            tc.tile_pool(name="pacc", bufs=2, space="PSUM") as pacc,
        ):
            # --- identity (f16, for PE bias-add matmuls) --------------------
            ident = const.tile([P, P], F32)
            make_identity(nc, ident[:])
            identh = const.tile([P, P], F16)
            nc.vector.tensor_copy(identh[:], ident[:])

            # --- weights ----------------------------------------------------
            w_sbs = {}
            for name, d in (("wq", wq_d), ("wk", wk_d), ("wv", wv_d), ("wg", wg_d)):
                w_sb = const.tile([P, 2, 2 * C_H], F16, name=f"{name}_sb")
                nc.sync.dma_start(w_sb[:], d.ap().rearrange("(o p) f -> p o f", p=P))
                w_sbs[name] = w_sb
            # wo_sb[h]: Wo_h duplicated at row bands 0-31 AND 64-95 (zeros
            # elsewhere) — the two bands contract the two q-chunk lanes of
            # the col-paired AV accumulators in a single K=128 projection.
            wo_sb = []
            for h in range(NH_LOC):
                t = const.tile([P, C_IN], F16, name=f"wo{h}_sb")
                nc.any.memset(t[:], 0.0)
                for qb in (0, 64):
                    nc.sync.dma_start(
                        t[qb : qb + C_H, :], wo_d.ap()[h * C_H : (h + 1) * C_H, :]
                    )
                wo_sb.append(t)
            bg_sb = []
            for h in range(NH_LOC):
                t = const.tile([C_H, 1], F32, name=f"bg{h}_sb")
                nc.sync.dma_start(t[:], bg_d.ap()[h * C_H : (h + 1) * C_H, None])
                bg_sb.append(t)

            # --- x (already [ci, n] f16 from host) --------------------------
            xqT = const.tile([P, 2, N], F16)
            xkvT = const.tile([P, 2, N], F16)
            for x_d, xT in ((xqT_d, xqT), (xkvT_d, xkvT)):
                nc.sync.dma_start(xT[:], x_d.ap().rearrange("(o p) n -> p o n", p=P))

            # --- q/k projections -> K=128-padded [128, n] f16 ---------------
            # qTz: heads at rows 0-63, zeros below; kTz_h: only head h's 32
            # rows nonzero.  QK then runs with a dense K=128 contraction so
            # the PE HAM activity monitor sees it as busy (K<128 matmuls
            # don't count and the PE gets clock-throttled to 1.2 GHz).
            qTz = const.tile([P, N], F16)
            kTz = [const.tile([P, N], F16, name=f"ktz{h}") for h in range(NH_LOC)]
            nc.any.memset(qTz[:], 0.0)
            for h in range(NH_LOC):
                nc.any.memset(kTz[h][:], 0.0)
            for xT_src, wname in ((xqT, "wq"), (xkvT, "wk")):
                for nb in range(2):
                    pp = pbig.tile([2 * C_H, QW], F32, tag="pbig")
                    for cb in range(2):
                        for ns in range(2):
                            sl = slice(nb * QW + ns * 512, nb * QW + (ns + 1) * 512)
                            nc.tensor.matmul(
                                pp[:, ns * 512 : (ns + 1) * 512],
                                w_sbs[wname][:, cb, :],
                                xT_src[:, cb, sl],
                                start=(cb == 0),
                                stop=(cb == 1),
                            )
                    nsl_full = slice(nb * QW, (nb + 1) * QW)
                    if wname == "wq":
                        nc.vector.tensor_copy(qTz[: 2 * C_H, nsl_full], pp[:])
                    else:
                        nc.vector.tensor_copy(kTz[0][:C_H, nsl_full], pp[:C_H])
                        nc.vector.tensor_copy(
                            kTz[1][C_H : 2 * C_H, nsl_full], pp[C_H : 2 * C_H]
                        )

            # --- gate: sigmoid(q_x @ Wg + bg); row-replicated to 64-95 ---
            gTh = []
            for h in range(NH_LOC):
                g = const.tile([96, N], F32, name=f"g{h}_sb")
                gTh.append(g)
                for nb in range(2):
                    pg = pbig.tile([C_H, QW], F32, tag="pbig")
                    for cb in range(2):
                        for ns in range(2):
                            sl = slice(nb * QW + ns * 512, nb * QW + (ns + 1) * 512)
                            nc.tensor.matmul(
                                pg[:, ns * 512 : (ns + 1) * 512],
                                w_sbs["wg"][:, cb, h * C_H : (h + 1) * C_H],
                                xqT[:, cb, sl],
                                start=(cb == 0),
                                stop=(cb == 1),
                            )
                    nc.scalar.activation(
                        g[:C_H, nb * QW : (nb + 1) * QW],
                        pg[:],
                        mybir.ActivationFunctionType.Sigmoid,
                        bias=bg_sb[h][:C_H],
                    )
                # replicate rows 0-31 -> 64-95 (for the qc1 lane band)
                nc.sync.dma_start(g[64:96, :], g[:C_H, :])

            # --- V' = [V | ones] per head: [k(128) x 16, 33] f16 ------------
            Vp = []
            for h in range(NH_LOC):
                v = const.tile([P, KC, 34], F16, name=f"vp{h}_sb")
                nc.any.memset(v[:], V_SCALE)
                Vp.append(v)
            for h in range(NH_LOC):
                for kc in range(KC):
                    pv = pacc.tile([P, 64], F32, tag="pacc")
                    for cb in range(2):
                        nc.tensor.matmul(
                            pv[:, :C_H],
                            xkvT[:, cb, kc * P : (kc + 1) * P],
                            w_sbs["wv"][:, cb, h * C_H : (h + 1) * C_H],
                            start=(cb == 0),
                            stop=(cb == 1),
                        )
                    nc.vector.tensor_copy(Vp[h][:, kc, :C_H], pv[:, :C_H])

            # --- main attention loop (head-sequential; q-chunks col-paired) -
            # oFTz [128, N]: qc0 data at rows 0-31, qc1 data at rows 64-95,
            # zeros elsewhere; wo_sb has Wo_h at BOTH row bands, so one
            # K=128 projection handles both column halves.
            oFT = []
            for h in range(NH_LOC):
                o = const.tile([P, N], F16, name=f"oft{h}_sb")
                nc.any.memset(o[:], 0.0)
                oFT.append(o)
            sums_sb = const.tile([P, NH_LOC, N], F32)
            bias_rr = [bias_d.ap()[h].rearrange("(o p) q -> p o q", p=P)
                       for h in range(NH_LOC)]
            QB = [0, 64]  # lane base per q-chunk

            for h in range(NH_LOC):
                oa0 = pacc.tile([33, QW], F32, tag="pacc", name=f"oa0_{h}")
                oa1 = pacc.tile([97, QW], F32, tag="pacc", name=f"oa1_{h}")
                oaccs = [oa0, oa1]
                for kc2 in range(KC // 2):
                    bt = work.tile([P, 2, N], F16, tag="bias", bufs=3)
                    nc.sync.dma_start(bt[:], bias_rr[h][:, 2 * kc2 : 2 * kc2 + 2, :])
                    for kcl in range(2):
                        kc = kc2 * 2 + kcl
                        ksl = slice(kc * P, (kc + 1) * P)
                        prs = []
                        for qc in range(2):
                            ps = pbig.tile([P, QW], F32, tag="pbig", name=f"ps{qc}")
                            for ns in range(2):
                                nsl = slice(ns * 512, (ns + 1) * 512)
                                gsl = slice(qc * QW + ns * 512,
                                            qc * QW + (ns + 1) * 512)
                                nc.tensor.matmul(
                                    ps[:, nsl],
                                    kTz[h][:, ksl],
                                    qTz[:, gsl],
                                    start=True,
                                    stop=False,
                                )
                                nc.tensor.matmul(
                                    ps[:, nsl],
                                    identh[:],
                                    bt[:, kcl, gsl],
                                    start=False,
                                    stop=True,
                                )
                            pr = work.tile([P, QW], F16, tag="probs",
                                           name=f"pr{qc}", bufs=4)
                            nc.scalar.activation(
                                pr[:], ps[:], mybir.ActivationFunctionType.Exp
                            )
                            prs.append(pr)
                        # AV: both q-chunks concurrently via PE column tiling
                        for ns in range(2):
                            nsl = slice(ns * 512, (ns + 1) * 512)
                            for qc in range(2):
                                nc.tensor.matmul(
                                    oaccs[qc][QB[qc] : QB[qc] + 33, nsl],
                                    Vp[h][:, kc, :33],
                                    prs[qc][:, nsl],
                                    start=(kc == 0),
                                    stop=(kc == KC - 1),
                                )
                # epilogue + output projection for this head (overlaps the
                # next head's main loop)
                for qc in range(2):
                    qsl = slice(qc * QW, (qc + 1) * QW)
                    sr = QB[qc] + 32
                    nc.vector.tensor_copy(
                        sums_sb[sr : sr + 1, h, qsl], oaccs[qc][sr : sr + 1, :]
                    )
                    nc.vector.tensor_tensor(
                        oFT[h][QB[qc] : QB[qc] + C_H, qsl],
                        oaccs[qc][QB[qc] : QB[qc] + C_H, :],
                        gTh[h][QB[qc] : QB[qc] + C_H, qsl],
                        mybir.AluOpType.mult,
                    )
                for cb in range(2):
                    ob = work.tile([P, N], F16, tag="oproj", bufs=2)
                    for nb in range(4):
                        po = pbig.tile([P, 512], F32, tag="pbig")
                        nc.tensor.matmul(
                            po[:],
                            wo_sb[h][:, cb * P : (cb + 1) * P],
                            oFT[h][:, nb * 512 : (nb + 1) * 512],
                            start=True,
                            stop=True,
                        )
                        nc.any.tensor_copy(ob[:, nb * 512 : (nb + 1) * 512], po[:])
                    nc.sync.dma_start(outp_d.ap()[h, cb], ob[:])
                for qc in range(2):
                    nc.sync.dma_start(
                        sums_d.ap()[0, h, qc * QW : (qc + 1) * QW, None],
                        sums_sb[QB[qc] + 32 : QB[qc] + 33, h,
                                qc * QW : (qc + 1) * QW],
                    )

    nc.compile()
    return nc


_NC_CACHE = None
LAST_RESULTS = None


def _get_nc():
    global _NC_CACHE
    if _NC_CACHE is None:
        _NC_CACHE = build_nc()
    return _NC_CACHE


def make_in_maps(q_x, kv_x, bias, Wq, Wk, Wv, Wg, bg, Wo):
    inv = 1.0 / math.sqrt(C_H)
    q_x = np.asarray(q_x, np.float32)
    kv_x = np.asarray(kv_x, np.float32)
    wq16 = (np.asarray(Wq, np.float32) * inv).astype(np.float16)
    wk16 = np.asarray(Wk, np.float32).astype(np.float16)
    wv16 = (np.asarray(Wv, np.float32) * V_SCALE).astype(np.float16)
    wg16 = np.asarray(Wg, np.float32).astype(np.float16)
    wo16 = np.asarray(Wo, np.float32).astype(np.float16)
    bg32 = np.asarray(bg, np.float32)
    # pre-transpose bias to [b, h, k, q] so the device loads it with plain
    # contiguous DMA (fp32 can't use the xbar DMA transpose; this also
    # avoids the costly per-call DMA_TRANSPOSE dispatch on the Sync engine)
    bias16 = np.ascontiguousarray(
        np.asarray(bias).astype(np.float16).transpose(0, 1, 3, 2)
    )
    xqT16 = [np.ascontiguousarray(q_x[b].T.astype(np.float16)) for b in range(B)]
    xkvT16 = [np.ascontiguousarray(kv_x[b].T.astype(np.float16)) for b in range(B)]

    in_maps = []
    for c in range(8):
        b, hp = c // 4, c % 4
        h0 = hp * NH_LOC
        cs = slice(h0 * C_H, (h0 + NH_LOC) * C_H)
        in_maps.append(
            {
                "xqT": xqT16[b],
                "xkvT": xkvT16[b],
                "biasf": np.ascontiguousarray(bias16[b, h0 : h0 + NH_LOC]),
                "wq": np.ascontiguousarray(wq16[:, cs]),
                "wk": np.ascontiguousarray(wk16[:, cs]),
                "wv": np.ascontiguousarray(wv16[:, cs]),
                "wg": np.ascontiguousarray(wg16[:, cs]),
                "wo": np.ascontiguousarray(wo16[cs, :]),
                "bg": np.ascontiguousarray(bg32[cs]),
            }
        )
    return in_maps


def assemble(results, bo):
    """Combine per-core outputs: divide by softmax sums, sum head pairs, + bo."""
    out = np.zeros((B, C_IN, N), np.float32)
    for c in range(8):
        b = c // 4
        outp = np.asarray(results[c]["outp"], np.float32)  # [NH_LOC, 2, P, N]
        sums = np.asarray(results[c]["sums"], np.float32).reshape(NH_LOC, N)
        for h in range(NH_LOC):
            out[b] += outp[h].reshape(C_IN, N) / sums[h][None, :]
    out = out.transpose(0, 2, 1) + np.asarray(bo, np.float32)[None, None, :]
    return np.ascontiguousarray(out)


def kernel(q_x, kv_x, bias, Wq, Wk, Wv, Wg, bg, Wo, bo, **run_kwargs):
    global LAST_RESULTS
    from concourse.bass_utils import run_bass_kernel_spmd

    nc = _get_nc()
    in_maps = make_in_maps(q_x, kv_x, bias, Wq, Wk, Wv, Wg, bg, Wo)
    res = run_bass_kernel_spmd(nc, in_maps, core_ids=list(range(8)), **run_kwargs)
    LAST_RESULTS = res
    return assemble(res.results, bo)


# revision 30
# speedup vs baseline: 1.0253x; 1.0253x over previous
"""Trainium2 Bass kernel for biased multi-head attention with sigmoid gating.

Problem (B=2, N=2048, C_IN=256, H=8, C_H=32):
    q = (q_x @ Wq) / sqrt(C_H);  k = kv_x @ Wk;  v = kv_x @ Wv
    a = softmax(q k^T + bias);   o = (a v) * sigmoid(q_x @ Wg + bg)
    out = o @ Wo + bo

Sharding: 8 cores, each takes (batch b = core//4, head pair hp = core%4).
Per core the kernel computes, for its 2 heads, the *unnormalized* gated
attention output projected through Wo, plus the softmax denominators; the
host divides by the denominators, sums partials over head-pairs, and adds bo.

Device-side layout highlights:
  - everything enters the PE in float16 (1 cycle/row vs 4 for fp32)
  - x and bias arrive host-pre-transposed f16, so no on-chip transposes
  - scores are built transposed [k, q] in PSUM: a K=128 zero-padded QK^T
    matmul (full-density contraction keeps the PE HAM activity monitor
    happy -- K<128 matmuls don't count as "busy" and the PE clock drops
    from 2.4 to 1.2 GHz), then the bias tile is accumulated into the same
    PSUM bank by an identity-weight matmul
  - softmax denominator comes free from a ones-column appended to V
  - exp runs on ScalarE straight out of PSUM, writing f16 probs to SBUF
  - the two 1024-wide q-chunks' AV matmuls are column-tiled to PE
    partition bases 0 and 64 and run concurrently; Wo is duplicated at
    both row bands so one K=128 projection covers both lanes
"""

import math
import sys

import numpy as np

sys.path.insert(0, "/opt/trn_rl_repo")

import concourse.bass as bass  # noqa: E402
import concourse.mybir as mybir  # noqa: E402
import concourse.tile as tile  # noqa: E402
from concourse import bacc  # noqa: E402
from concourse.masks import make_identity  # noqa: E402

B, N, C_IN = 2, 2048, 256
H, C_H = 8, 32
P = 128
NH_LOC = 2  # heads per core
QW = 1024  # q-chunk width in the main loop
KC = N // P  # 16 k-chunks per head
V_SCALE = 1.0 / 64.0  # keeps unnormalized (exp @ V) in f16 range; cancels on host
F32 = mybir.dt.float32
F16 = mybir.dt.float16


def build_nc():
    nc = bacc.Bacc("TRN2", target_bir_lowering=False, debug=False)

    xqT_d = nc.dram_tensor("xqT", [C_IN, N], F16, kind="ExternalInput")
    xkvT_d = nc.dram_tensor("xkvT", [C_IN, N], F16, kind="ExternalInput")
    bias_d = nc.dram_tensor("biasf", [NH_LOC, N, N], F16, kind="ExternalInput")
    wq_d = nc.dram_tensor("wq", [C_IN, 2 * C_H], F16, kind="ExternalInput")
    wk_d = nc.dram_tensor("wk", [C_IN, 2 * C_H], F16, kind="ExternalInput")
    wv_d = nc.dram_tensor("wv", [C_IN, 2 * C_H], F16, kind="ExternalInput")
    wg_d = nc.dram_tensor("wg", [C_IN, 2 * C_H], F16, kind="ExternalInput")
    wo_d = nc.dram_tensor("wo", [2 * C_H, C_IN], F16, kind="ExternalInput")
    bg_d = nc.dram_tensor("bg", [2 * C_H], F32, kind="ExternalInput")
    outp_d = nc.dram_tensor("outp", [NH_LOC, 2, P, N], F16, kind="ExternalOutput")
    sums_d = nc.dram_tensor("sums", [1, NH_LOC, N], F32, kind="ExternalOutput")

    with tile.TileContext(nc) as tc:
        with (
            tc.tile_pool(name="const", bufs=1) as const,
            tc.tile_pool(name="work", bufs=3) as work,
            tc.tile_pool(name="pbig", bufs=2, space="PSUM") as pbig,
            tc.tile_pool(name="pacc", bufs=2, space="PSUM") as pacc,
        ):
            # --- x first: the projections gate the whole pipeline ----------
            xqT = const.tile([P, 2, N], F16)
            xkvT = const.tile([P, 2, N], F16)
            for x_d, xT in ((xqT_d, xqT), (xkvT_d, xkvT)):
                nc.sync.dma_start(xT[:], x_d.ap().rearrange("(o p) n -> p o n", p=P))

            # --- identity (f16, for PE bias-add matmuls) --------------------
            ident = const.tile([P, P], F32)
            make_identity(nc, ident[:])
            identh = const.tile([P, P], F16)
            nc.vector.tensor_copy(identh[:], ident[:])

            # --- weights (SWDGE queue so they don't serialize behind the
            # big Sync-queue transfers) --------------------------------------
            w_sbs = {}
            for name, d in (("wq", wq_d), ("wk", wk_d), ("wv", wv_d), ("wg", wg_d)):
                w_sb = const.tile([P, 2, 2 * C_H], F16, name=f"{name}_sb")
                nc.gpsimd.dma_start(w_sb[:], d.ap().rearrange("(o p) f -> p o f", p=P))
                w_sbs[name] = w_sb
            # wo_sb[h]: Wo_h duplicated at row bands 0-31 AND 64-95 (zeros
            # elsewhere) — the two bands contract the two q-chunk lanes of
            # the col-paired AV accumulators in a single K=128 projection.
            wo_sb = []
            for h in range(NH_LOC):
                t = const.tile([P, C_IN], F16, name=f"wo{h}_sb")
                nc.any.memset(t[:], 0.0)
                for qb in (0, 64):
                    nc.gpsimd.dma_start(
                        t[qb : qb + C_H, :], wo_d.ap()[h * C_H : (h + 1) * C_H, :]
                    )
                wo_sb.append(t)
            bg_sb = []
            for h in range(NH_LOC):
                t = const.tile([C_H, 1], F32, name=f"bg{h}_sb")
                nc.gpsimd.dma_start(t[:], bg_d.ap()[h * C_H : (h + 1) * C_H, None])
                bg_sb.append(t)

            # --- q/k projections -> K=128-padded [128, n] f16 ---------------
            # qTz: heads at rows 0-63, zeros below; kTz_h: only head h's 32
            # rows nonzero.  QK then runs with a dense K=128 contraction so
            # the PE HAM activity monitor sees it as busy (K<128 matmuls
            # don't count and the PE gets clock-throttled to 1.2 GHz).
            qTz = const.tile([P, N], F16)
            kTz = [const.tile([P, N], F16, name=f"ktz{h}") for h in range(NH_LOC)]
            nc.any.memset(qTz[:], 0.0)
            for h in range(NH_LOC):
                nc.any.memset(kTz[h][:], 0.0)
            for xT_src, wname in ((xqT, "wq"), (xkvT, "wk")):
                for nb in range(2):
                    pp = pbig.tile([2 * C_H, QW], F32, tag="pbig")
                    for cb in range(2):
                        for ns in range(2):
                            sl = slice(nb * QW + ns * 512, nb * QW + (ns + 1) * 512)
                            nc.tensor.matmul(
                                pp[:, ns * 512 : (ns + 1) * 512],
                                w_sbs[wname][:, cb, :],
                                xT_src[:, cb, sl],
                                start=(cb == 0),
                                stop=(cb == 1),
                            )
                    nsl_full = slice(nb * QW, (nb + 1) * QW)
                    if wname == "wq":
                        nc.vector.tensor_copy(qTz[: 2 * C_H, nsl_full], pp[:])
                    else:
                        nc.vector.tensor_copy(kTz[0][:C_H, nsl_full], pp[:C_H])
                        nc.vector.tensor_copy(
                            kTz[1][C_H : 2 * C_H, nsl_full], pp[C_H : 2 * C_H]
                        )

            # --- gate: sigmoid(q_x @ Wg + bg); row-replicated to 64-95 ---
            gTh = []
            for h in range(NH_LOC):
                g = const.tile([96, N], F32, name=f"g{h}_sb")
                gTh.append(g)
                for nb in range(2):
                    pg = pbig.tile([C_H, QW], F32, tag="pbig")
                    for cb in range(2):
                        for ns in range(2):
                            sl = slice(nb * QW + ns * 512, nb * QW + (ns + 1) * 512)
                            nc.tensor.matmul(
                                pg[:, ns * 512 : (ns + 1) * 512],
                                w_sbs["wg"][:, cb, h * C_H : (h + 1) * C_H],
                                xqT[:, cb, sl],
                                start=(cb == 0),
                                stop=(cb == 1),
                            )
                    nc.scalar.activation(
                        g[:C_H, nb * QW : (nb + 1) * QW],
                        pg[:],
                        mybir.ActivationFunctionType.Sigmoid,
                        bias=bg_sb[h][:C_H],
                    )
                # replicate rows 0-31 -> 64-95 (for the qc1 lane band)
                nc.sync.dma_start(g[64:96, :], g[:C_H, :])

            # --- V' = [V | ones] per head: [k(128) x 16, 33] f16 ------------
            Vp = []
            for h in range(NH_LOC):
                v = const.tile([P, KC, 34], F16, name=f"vp{h}_sb")
                nc.any.memset(v[:], V_SCALE)
                Vp.append(v)
            for h in range(NH_LOC):
                for kc in range(KC):
                    pv = pacc.tile([P, 64], F32, tag="pacc")
                    for cb in range(2):
                        nc.tensor.matmul(
                            pv[:, :C_H],
                            xkvT[:, cb, kc * P : (kc + 1) * P],
                            w_sbs["wv"][:, cb, h * C_H : (h + 1) * C_H],
                            start=(cb == 0),
                            stop=(cb == 1),
                        )
                    nc.vector.tensor_copy(Vp[h][:, kc, :C_H], pv[:, :C_H])

            # --- main attention loop (head-sequential; q-chunks col-paired) -
            # oFTz [128, N]: qc0 data at rows 0-31, qc1 data at rows 64-95,
            # zeros elsewhere; wo_sb has Wo_h at BOTH row bands, so one
            # K=128 projection handles both column halves.
            oFT = []
            for h in range(NH_LOC):
                o = const.tile([P, N], F16, name=f"oft{h}_sb")
                nc.any.memset(o[:], 0.0)
                oFT.append(o)
            sums_sb = const.tile([P, NH_LOC, N], F32)
            bias_rr = [bias_d.ap()[h].rearrange("(o p) q -> p o q", p=P)
                       for h in range(NH_LOC)]
            QB = [0, 64]  # lane base per q-chunk

            for h in range(NH_LOC):
                oa0 = pacc.tile([33, QW], F32, tag="pacc", name=f"oa0_{h}")
                oa1 = pacc.tile([97, QW], F32, tag="pacc", name=f"oa1_{h}")
                oaccs = [oa0, oa1]
                for kc2 in range(KC // 2):
                    bt = work.tile([P, 2, N], F16, tag="bias", bufs=3)
                    nc.sync.dma_start(bt[:], bias_rr[h][:, 2 * kc2 : 2 * kc2 + 2, :])
                    for kcl in range(2):
                        kc = kc2 * 2 + kcl
                        ksl = slice(kc * P, (kc + 1) * P)
                        prs = []
                        for qc in range(2):
                            ps = pbig.tile([P, QW], F32, tag="pbig", name=f"ps{qc}")
                            for ns in range(2):
                                nsl = slice(ns * 512, (ns + 1) * 512)
                                gsl = slice(qc * QW + ns * 512,
                                            qc * QW + (ns + 1) * 512)
                                nc.tensor.matmul(
                                    ps[:, nsl],
                                    kTz[h][:, ksl],
                                    qTz[:, gsl],
                                    start=True,
                                    stop=False,
                                )
                                nc.tensor.matmul(
                                    ps[:, nsl],
                                    identh[:],
                                    bt[:, kcl, gsl],
                                    start=False,
                                    stop=True,
                                )
                            pr = work.tile([P, QW], F16, tag="probs",
                                           name=f"pr{qc}", bufs=4)
                            nc.scalar.activation(
                                pr[:], ps[:], mybir.ActivationFunctionType.Exp
                            )
                            prs.append(pr)
                        # AV: both q-chunks concurrently via PE column tiling
                        for ns in range(2):
                            nsl = slice(ns * 512, (ns + 1) * 512)
                            for qc in range(2):
                                nc.tensor.matmul(
                                    oaccs[qc][QB[qc] : QB[qc] + 33, nsl],
                                    Vp[h][:, kc, :33],
                                    prs[qc][:, nsl],
                                    start=(kc == 0),
                                    stop=(kc == KC - 1),
                                )
                # epilogue + output projection for this head (overlaps the
                # next head's main loop)
                for qc in range(2):
                    qsl = slice(qc * QW, (qc + 1) * QW)
                    sr = QB[qc] + 32
                    nc.vector.tensor_copy(
                        sums_sb[sr : sr + 1, h, qsl], oaccs[qc][sr : sr + 1, :]
                    )
                    nc.vector.tensor_tensor(
                        oFT[h][QB[qc] : QB[qc] + C_H, qsl],
                        oaccs[qc][QB[qc] : QB[qc] + C_H, :],
                        gTh[h][QB[qc] : QB[qc] + C_H, qsl],
                        mybir.AluOpType.mult,
                    )
                for cb in range(2):
                    ob = work.tile([P, N], F16, tag="oproj", bufs=2)
                    for nb in range(4):
                        po = pbig.tile([P, 512], F32, tag="pbig")
                        nc.tensor.matmul(
                            po[:],
                            wo_sb[h][:, cb * P : (cb + 1) * P],
                            oFT[h][:, nb * 512 : (nb + 1) * 512],
                            start=True,
                            stop=True,
                        )
                        nc.any.tensor_copy(ob[:, nb * 512 : (nb + 1) * 512], po[:])
                    nc.sync.dma_start(outp_d.ap()[h, cb], ob[:])
                for qc in range(2):
                    nc.sync.dma_start(
                        sums_d.ap()[0, h, qc * QW : (qc + 1) * QW, None],
                        sums_sb[QB[qc] + 32 : QB[qc] + 33, h,
                                qc * QW : (qc + 1) * QW],
                    )

    nc.compile()
    return nc


_NC_CACHE = None
LAST_RESULTS = None


def _get_nc():
    global _NC_CACHE
    if _NC_CACHE is None:
        _NC_CACHE = build_nc()
    return _NC_CACHE


def make_in_maps(q_x, kv_x, bias, Wq, Wk, Wv, Wg, bg, Wo):
    inv = 1.0 / math.sqrt(C_H)
    q_x = np.asarray(q_x, np.float32)
    kv_x = np.asarray(kv_x, np.float32)
    wq16 = (np.asarray(Wq, np.float32) * inv).astype(np.float16)
    wk16 = np.asarray(Wk, np.float32).astype(np.float16)
    wv16 = (np.asarray(Wv, np.float32) * V_SCALE).astype(np.float16)
    wg16 = np.asarray(Wg, np.float32).astype(np.float16)
    wo16 = np.asarray(Wo, np.float32).astype(np.float16)
    bg32 = np.asarray(bg, np.float32)
    # pre-transpose bias to [b, h, k, q] so the device loads it with plain
    # contiguous DMA (fp32 can't use the xbar DMA transpose; this also
    # avoids the costly per-call DMA_TRANSPOSE dispatch on the Sync engine)
    bias16 = np.ascontiguousarray(
        np.asarray(bias).astype(np.float16).transpose(0, 1, 3, 2)
    )
    xqT16 = [np.ascontiguousarray(q_x[b].T.astype(np.float16)) for b in range(B)]
    xkvT16 = [np.ascontiguousarray(kv_x[b].T.astype(np.float16)) for b in range(B)]

    in_maps = []
    for c in range(8):
        b, hp = c // 4, c % 4
        h0 = hp * NH_LOC
        cs = slice(h0 * C_H, (h0 + NH_LOC) * C_H)
        in_maps.append(
            {
                "xqT": xqT16[b],
                "xkvT": xkvT16[b],
                "biasf": np.ascontiguousarray(bias16[b, h0 : h0 + NH_LOC]),
                "wq": np.ascontiguousarray(wq16[:, cs]),
                "wk": np.ascontiguousarray(wk16[:, cs]),
                "wv": np.ascontiguousarray(wv16[:, cs]),
                "wg": np.ascontiguousarray(wg16[:, cs]),
                "wo": np.ascontiguousarray(wo16[cs, :]),
                "bg": np.ascontiguousarray(bg32[cs]),
            }
        )
    return in_maps


def assemble(results, bo):
    """Combine per-core outputs: divide by softmax sums, sum head pairs, + bo."""
    out = np.zeros((B, C_IN, N), np.float32)
    for c in range(8):
        b = c // 4
        outp = np.asarray(results[c]["outp"], np.float32)  # [NH_LOC, 2, P, N]
        sums = np.asarray(results[c]["sums"], np.float32).reshape(NH_LOC, N)
        for h in range(NH_LOC):
            out[b] += outp[h].reshape(C_IN, N) / sums[h][None, :]
    out = out.transpose(0, 2, 1) + np.asarray(bo, np.float32)[None, None, :]
    return np.ascontiguousarray(out)


def kernel(q_x, kv_x, bias, Wq, Wk, Wv, Wg, bg, Wo, bo, **run_kwargs):
    global LAST_RESULTS
    from concourse.bass_utils import run_bass_kernel_spmd

    nc = _get_nc()
    in_maps = make_in_maps(q_x, kv_x, bias, Wq, Wk, Wv, Wg, bg, Wo)
    res = run_bass_kernel_spmd(nc, in_maps, core_ids=list(range(8)), **run_kwargs)
    LAST_RESULTS = res
    return assemble(res.results, bo)


# revision 31
# speedup vs baseline: 1.0869x; 1.0601x over previous
"""Trainium2 Bass kernel for biased multi-head attention with sigmoid gating.

Problem (B=2, N=2048, C_IN=256, H=8, C_H=32):
    q = (q_x @ Wq) / sqrt(C_H);  k = kv_x @ Wk;  v = kv_x @ Wv
    a = softmax(q k^T + bias);   o = (a v) * sigmoid(q_x @ Wg + bg)
    out = o @ Wo + bo

Sharding: 8 cores, each takes (batch b = core//4, head pair hp = core%4).
Per core the kernel computes, for its 2 heads, the *unnormalized* gated
attention output projected through Wo, plus the softmax denominators; the
host divides by the denominators, sums partials over head-pairs, and adds bo.

Device-side layout highlights:
  - everything enters the PE in float16 (1 cycle/row vs 4 for fp32)
  - x and bias arrive host-pre-transposed f16, so no on-chip transposes
  - scores are built transposed [k, q] in PSUM: a K=128 zero-padded QK^T
    matmul (full-density contraction keeps the PE HAM activity monitor
    happy -- K<128 matmuls don't count as "busy" and the PE clock drops
    from 2.4 to 1.2 GHz), then the bias tile is accumulated into the same
    PSUM bank by an identity-weight matmul
  - softmax denominator comes free from a ones-column appended to V
  - exp runs on ScalarE straight out of PSUM, writing f16 probs to SBUF
  - the two 1024-wide q-chunks' AV matmuls are column-tiled to PE
    partition bases 0 and 64 and run concurrently; Wo is duplicated at
    both row bands so one K=128 projection covers both lanes
"""

import math
import sys

import numpy as np

sys.path.insert(0, "/opt/trn_rl_repo")

import concourse.bass as bass  # noqa: E402
import concourse.mybir as mybir  # noqa: E402
import concourse.tile as tile  # noqa: E402
from concourse import bacc  # noqa: E402
from concourse.masks import make_identity  # noqa: E402

B, N, C_IN = 2, 2048, 256
H, C_H = 8, 32
P = 128
NH_LOC = 2  # heads per core
QW = 1024  # q-chunk width in the main loop
KC = N // P  # 16 k-chunks per head
V_SCALE = 1.0 / 64.0  # keeps unnormalized (exp @ V) in f16 range; cancels on host
F32 = mybir.dt.float32
F16 = mybir.dt.float16


def build_nc():
    nc = bacc.Bacc("TRN2", target_bir_lowering=False, debug=False)

    xqT_d = nc.dram_tensor("xqT", [C_IN, N], F16, kind="ExternalInput")
    xkvT_d = nc.dram_tensor("xkvT", [C_IN, N], F16, kind="ExternalInput")
    bias_d = nc.dram_tensor("biasf", [NH_LOC, N, N], F16, kind="ExternalInput")
    wq_d = nc.dram_tensor("wq", [C_IN, 2 * C_H], F16, kind="ExternalInput")
    wk_d = nc.dram_tensor("wk", [C_IN, 2 * C_H], F16, kind="ExternalInput")
    wv_d = nc.dram_tensor("wv", [C_IN, 2 * C_H], F16, kind="ExternalInput")
    wg_d = nc.dram_tensor("wg", [C_IN, 2 * C_H], F16, kind="ExternalInput")
    wo_d = nc.dram_tensor("wo", [2 * C_H, C_IN], F16, kind="ExternalInput")
    bg_d = nc.dram_tensor("bg", [2 * C_H], F32, kind="ExternalInput")
    outp_d = nc.dram_tensor("outp", [NH_LOC, 2, P, N], F16, kind="ExternalOutput")
    sums_d = nc.dram_tensor("sums", [1, NH_LOC, N], F32, kind="ExternalOutput")

    with tile.TileContext(nc) as tc:
        with (
            tc.tile_pool(name="const", bufs=1) as const,
            tc.tile_pool(name="work", bufs=3) as work,
            tc.tile_pool(name="pbig", bufs=2, space="PSUM") as pbig,
            tc.tile_pool(name="pacc", bufs=2, space="PSUM") as pacc,
        ):
            # --- x first: the projections gate the whole pipeline ----------
            xqT = const.tile([P, 2, N], F16)
            xkvT = const.tile([P, 2, N], F16)
            for x_d, xT in ((xqT_d, xqT), (xkvT_d, xkvT)):
                nc.sync.dma_start(xT[:], x_d.ap().rearrange("(o p) n -> p o n", p=P))

            # --- identity (f16, for PE bias-add matmuls) --------------------
            ident = const.tile([P, P], F32)
            make_identity(nc, ident[:])
            identh = const.tile([P, P], F16)
            nc.vector.tensor_copy(identh[:], ident[:])

            # --- weights (SWDGE queue so they don't serialize behind the
            # big Sync-queue transfers) --------------------------------------
            w_sbs = {}
            for name, d in (("wq", wq_d), ("wk", wk_d), ("wv", wv_d), ("wg", wg_d)):
                w_sb = const.tile([P, 2, 2 * C_H], F16, name=f"{name}_sb")
                nc.gpsimd.dma_start(w_sb[:], d.ap().rearrange("(o p) f -> p o f", p=P))
                w_sbs[name] = w_sb
            # wo_sb[h]: Wo_h duplicated at row bands 0-31 AND 64-95 (zeros
            # elsewhere) — the two bands contract the two q-chunk lanes of
            # the col-paired AV accumulators in a single K=128 projection.
            wo_sb = []
            for h in range(NH_LOC):
                t = const.tile([P, C_IN], F16, name=f"wo{h}_sb")
                nc.any.memset(t[:], 0.0)
                for qb in (0, 64):
                    nc.gpsimd.dma_start(
                        t[qb : qb + C_H, :], wo_d.ap()[h * C_H : (h + 1) * C_H, :]
                    )
                wo_sb.append(t)
            bg_sb = []
            for h in range(NH_LOC):
                t = const.tile([C_H, 1], F32, name=f"bg{h}_sb")
                nc.gpsimd.dma_start(t[:], bg_d.ap()[h * C_H : (h + 1) * C_H, None])
                bg_sb.append(t)

            # --- q/k projections -> K=128-padded [128, n] f16 ---------------
            # qTz: heads at rows 0-63, zeros below; kTz_h: only head h's 32
            # rows nonzero.  QK then runs with a dense K=128 contraction so
            # the PE HAM activity monitor sees it as busy (K<128 matmuls
            # don't count and the PE gets clock-throttled to 1.2 GHz).
            qTz = const.tile([P, N], F16)
            kTz = [const.tile([P, N], F16, name=f"ktz{h}") for h in range(NH_LOC)]
            nc.any.memset(qTz[:], 0.0)
            for h in range(NH_LOC):
                nc.any.memset(kTz[h][:], 0.0)
            for xT_src, wname in ((xqT, "wq"), (xkvT, "wk")):
                for nb in range(2):
                    pp = pbig.tile([2 * C_H, QW], F32, tag="pbig")
                    for cb in range(2):
                        for ns in range(2):
                            sl = slice(nb * QW + ns * 512, nb * QW + (ns + 1) * 512)
                            nc.tensor.matmul(
                                pp[:, ns * 512 : (ns + 1) * 512],
                                w_sbs[wname][:, cb, :],
                                xT_src[:, cb, sl],
                                start=(cb == 0),
                                stop=(cb == 1),
                            )
                    nsl_full = slice(nb * QW, (nb + 1) * QW)
                    if wname == "wq":
                        nc.vector.tensor_copy(qTz[: 2 * C_H, nsl_full], pp[:])
                    else:
                        nc.vector.tensor_copy(kTz[0][:C_H, nsl_full], pp[:C_H])
                        nc.vector.tensor_copy(
                            kTz[1][C_H : 2 * C_H, nsl_full], pp[C_H : 2 * C_H]
                        )

            # --- gate: sigmoid(q_x @ Wg + bg); row-replicated to 64-95 ---
            gTh = []
            for h in range(NH_LOC):
                g = const.tile([96, N], F32, name=f"g{h}_sb")
                gTh.append(g)
                for nb in range(2):
                    pg = pbig.tile([C_H, QW], F32, tag="pbig")
                    for cb in range(2):
                        for ns in range(2):
                            sl = slice(nb * QW + ns * 512, nb * QW + (ns + 1) * 512)
                            nc.tensor.matmul(
                                pg[:, ns * 512 : (ns + 1) * 512],
                                w_sbs["wg"][:, cb, h * C_H : (h + 1) * C_H],
                                xqT[:, cb, sl],
                                start=(cb == 0),
                                stop=(cb == 1),
                            )
                    nc.scalar.activation(
                        g[:C_H, nb * QW : (nb + 1) * QW],
                        pg[:],
                        mybir.ActivationFunctionType.Sigmoid,
                        bias=bg_sb[h][:C_H],
                    )
                # replicate rows 0-31 -> 64-95 (for the qc1 lane band)
                nc.sync.dma_start(g[64:96, :], g[:C_H, :])

            # --- V' = [V | ones] per head: [k(128) x 16, 33] f16 ------------
            Vp = []
            for h in range(NH_LOC):
                v = const.tile([P, KC, 34], F16, name=f"vp{h}_sb")
                nc.any.memset(v[:], V_SCALE)
                Vp.append(v)
            for h in range(NH_LOC):
                for kc in range(KC):
                    pv = pacc.tile([P, 64], F32, tag="pacc")
                    for cb in range(2):
                        nc.tensor.matmul(
                            pv[:, :C_H],
                            xkvT[:, cb, kc * P : (kc + 1) * P],
                            w_sbs["wv"][:, cb, h * C_H : (h + 1) * C_H],
                            start=(cb == 0),
                            stop=(cb == 1),
                        )
                    nc.vector.tensor_copy(Vp[h][:, kc, :C_H], pv[:, :C_H])

            # --- main attention loop (head-sequential; q-chunks col-paired) -
            # oFTz [128, N]: qc0 data at rows 0-31, qc1 data at rows 64-95,
            # zeros elsewhere; wo_sb has Wo_h at BOTH row bands, so one
            # K=128 projection handles both column halves.
            oFT = []
            for h in range(NH_LOC):
                o = const.tile([P, N], F16, name=f"oft{h}_sb")
                nc.any.memset(o[:], 0.0)
                oFT.append(o)
            sums_sb = const.tile([P, NH_LOC, N], F32)
            bias_rr = [bias_d.ap()[h].rearrange("(o p) q -> p o q", p=P)
                       for h in range(NH_LOC)]
            QB = [0, 64]  # lane base per q-chunk

            for h in range(NH_LOC):
                oa0 = pacc.tile([33, QW], F32, tag="pacc", name=f"oa0_{h}")
                oa1 = pacc.tile([97, QW], F32, tag="pacc", name=f"oa1_{h}")
                oaccs = [oa0, oa1]
                for kc2 in range(KC // 2):
                    bt = work.tile([P, 2, N], F16, tag="bias", bufs=3)
                    nc.sync.dma_start(bt[:], bias_rr[h][:, 2 * kc2 : 2 * kc2 + 2, :])
                    for kcl in range(2):
                        kc = kc2 * 2 + kcl
                        ksl = slice(kc * P, (kc + 1) * P)
                        prs = []
                        for qc in range(2):
                            ps = pbig.tile([P, QW], F32, tag="pbig", name=f"ps{qc}")
                            for ns in range(2):
                                nsl = slice(ns * 512, (ns + 1) * 512)
                                gsl = slice(qc * QW + ns * 512,
                                            qc * QW + (ns + 1) * 512)
                                nc.tensor.matmul(
                                    ps[:, nsl],
                                    kTz[h][:, ksl],
                                    qTz[:, gsl],
                                    start=True,
                                    stop=False,
                                )
                                nc.tensor.matmul(
                                    ps[:, nsl],
                                    identh[:],
                                    bt[:, kcl, gsl],
                                    start=False,
                                    stop=True,
                                )
                            pr = work.tile([P, QW], F16, tag="probs",
                                           name=f"pr{qc}", bufs=4)
                            nc.scalar.activation(
                                pr[:], ps[:], mybir.ActivationFunctionType.Exp
                            )
                            prs.append(pr)
                        # AV: both q-chunks concurrently via PE column tiling
                        for ns in range(2):
                            nsl = slice(ns * 512, (ns + 1) * 512)
                            for qc in range(2):
                                nc.tensor.matmul(
                                    oaccs[qc][QB[qc] : QB[qc] + 33, nsl],
                                    Vp[h][:, kc, :33],
                                    prs[qc][:, nsl],
                                    start=(kc == 0),
                                    stop=(kc == KC - 1),
                                )
                # epilogue + output projection for this head (overlaps the
                # next head's main loop)
                for qc in range(2):
                    qsl = slice(qc * QW, (qc + 1) * QW)
                    sr = QB[qc] + 32
                    nc.vector.tensor_copy(
                        sums_sb[sr : sr + 1, h, qsl], oaccs[qc][sr : sr + 1, :]
                    )
                    nc.vector.tensor_tensor(
                        oFT[h][QB[qc] : QB[qc] + C_H, qsl],
                        oaccs[qc][QB[qc] : QB[qc] + C_H, :],
                        gTh[h][QB[qc] : QB[qc] + C_H, qsl],
                        mybir.AluOpType.mult,
                    )
                for qc in range(2):
                    nc.sync.dma_start(
                        sums_d.ap()[0, h, qc * QW : (qc + 1) * QW, None],
                        sums_sb[QB[qc] + 32 : QB[qc] + 33, h,
                                qc * QW : (qc + 1) * QW],
                    )

            for h in range(NH_LOC):
                for cb in range(2):
                    ob = work.tile([P, N], F16, tag="oproj", bufs=2)
                    for nb in range(4):
                        po = pbig.tile([P, 512], F32, tag="pbig")
                        nc.tensor.matmul(
                            po[:],
                            wo_sb[h][:, cb * P : (cb + 1) * P],
                            oFT[h][:, nb * 512 : (nb + 1) * 512],
                            start=True,
                            stop=True,
                        )
                        nc.any.tensor_copy(ob[:, nb * 512 : (nb + 1) * 512], po[:])
                    nc.sync.dma_start(outp_d.ap()[h, cb], ob[:])

    nc.compile()
    return nc


_NC_CACHE = None
LAST_RESULTS = None


def _get_nc():
    global _NC_CACHE
    if _NC_CACHE is None:
        _NC_CACHE = build_nc()
    return _NC_CACHE


def make_in_maps(q_x, kv_x, bias, Wq, Wk, Wv, Wg, bg, Wo):
    inv = 1.0 / math.sqrt(C_H)
    q_x = np.asarray(q_x, np.float32)
    kv_x = np.asarray(kv_x, np.float32)
    wq16 = (np.asarray(Wq, np.float32) * inv).astype(np.float16)
    wk16 = np.asarray(Wk, np.float32).astype(np.float16)
    wv16 = (np.asarray(Wv, np.float32) * V_SCALE).astype(np.float16)
    wg16 = np.asarray(Wg, np.float32).astype(np.float16)
    wo16 = np.asarray(Wo, np.float32).astype(np.float16)
    bg32 = np.asarray(bg, np.float32)
    # pre-transpose bias to [b, h, k, q] so the device loads it with plain
    # contiguous DMA (fp32 can't use the xbar DMA transpose; this also
    # avoids the costly per-call DMA_TRANSPOSE dispatch on the Sync engine)
    bias16 = np.ascontiguousarray(
        np.asarray(bias).astype(np.float16).transpose(0, 1, 3, 2)
    )
    xqT16 = [np.ascontiguousarray(q_x[b].T.astype(np.float16)) for b in range(B)]
    xkvT16 = [np.ascontiguousarray(kv_x[b].T.astype(np.float16)) for b in range(B)]

    in_maps = []
    for c in range(8):
        b, hp = c // 4, c % 4
        h0 = hp * NH_LOC
        cs = slice(h0 * C_H, (h0 + NH_LOC) * C_H)
        in_maps.append(
            {
                "xqT": xqT16[b],
                "xkvT": xkvT16[b],
                "biasf": np.ascontiguousarray(bias16[b, h0 : h0 + NH_LOC]),
                "wq": np.ascontiguousarray(wq16[:, cs]),
                "wk": np.ascontiguousarray(wk16[:, cs]),
                "wv": np.ascontiguousarray(wv16[:, cs]),
                "wg": np.ascontiguousarray(wg16[:, cs]),
                "wo": np.ascontiguousarray(wo16[cs, :]),
                "bg": np.ascontiguousarray(bg32[cs]),
            }
        )
    return in_maps


def assemble(results, bo):
    """Combine per-core outputs: divide by softmax sums, sum head pairs, + bo."""
    out = np.zeros((B, C_IN, N), np.float32)
    for c in range(8):
        b = c // 4
        outp = np.asarray(results[c]["outp"], np.float32)  # [NH_LOC, 2, P, N]
        sums = np.asarray(results[c]["sums"], np.float32).reshape(NH_LOC, N)
        for h in range(NH_LOC):
            out[b] += outp[h].reshape(C_IN, N) / sums[h][None, :]
    out = out.transpose(0, 2, 1) + np.asarray(bo, np.float32)[None, None, :]
    return np.ascontiguousarray(out)


def kernel(q_x, kv_x, bias, Wq, Wk, Wv, Wg, bg, Wo, bo, **run_kwargs):
    global LAST_RESULTS
    from concourse.bass_utils import run_bass_kernel_spmd

    nc = _get_nc()
    in_maps = make_in_maps(q_x, kv_x, bias, Wq, Wk, Wv, Wg, bg, Wo)
    res = run_bass_kernel_spmd(nc, in_maps, core_ids=list(range(8)), **run_kwargs)
    LAST_RESULTS = res
    return assemble(res.results, bo)
